# revision 44
# baseline (speedup 1.0000x reference)
"""Trainium2 Bass kernel for nn_ConstrainedEnhancementModel.

Model: x(512,256,32) -> flatten(512,8192) -> MLP encoder/decoder
(8192->1024->512->256->512->1024->131072) -> constraint blend with
linearly-interpolated low-res data.

Key transformation (host-side fold): the post-matmul constraint
   out = where(knot, x_knot, where(in_seg, 0.8*interp + 0.2*decoded, decoded))
is linear in x, so it folds into the final projection:
 - scale w6/b6 columns by 0 (knot), 0.2 (in-segment) or 1.0 (tail)
 - per 512-column segment, append a K=65 matmul block:
   32 rows of x (segment start vals), 32 rows (segment end vals), and a
   ones-row carrying the bias b6_eff.
Device kernel is then a pure matmul pipeline (no gather/select on chip).

Precision plan (rel-err budget 2e-2): the dominant projection runs in
fp8 e4m3 with MatmulPerfMode.DoubleRow (K=256/instr, 2x PE rate).  w6 is
pre-scaled by 32 so its N(0, 1/32) entries use e4m3's normal range; the
interp coefficient block is pre-scaled by 32 in bf16, and the PSUM is
descaled by exactly 1/32 during evacuation.  decoded carries only 0.2
weight in most of the output, so its ~4% fp8 error contributes ~1%.
Encoder runs in bf16 end-to-end (including the AllReduce payload).
Output is written bf16 and upcast on host; knot columns are overwritten
host-side with exact f32 input values.

Sharding over 8 cores: encoder L1 is K-sharded + AllReduce at full
batch in transposed layout [features, batch]; L2-L5 replicated; the
1024x131072 projection is tensor-parallel column-sharded (16384 cols =
512 output timesteps per core); core c writes output rows [c*512,
(c+1)*512) of H.

Perf notes: DMA instruction issue costs ~0.6-1us of sequencer time, so
everything is loaded with few, large multi-dim-AP DMAs split across the
two HWDGE-capable engines (sync, scalar).
"""

from contextlib import ExitStack

import numpy as np
from ml_dtypes import bfloat16, float8_e4m3

import concourse.bacc as bacc
import concourse.mybir as mybir
import concourse.tile as tile
from concourse.bass import ds, ts
from concourse.bass_utils import run_bass_kernel_spmd

DT = mybir.dt

B, L, F, H, HID = 512, 256, 32, 4096, 512
UP = H // L          # 16 timesteps per low-res segment
LF = L * F           # 8192
HF = H * F           # 131072
NCORES = 8
COLS = HF // NCORES  # 16384 output cols per core
SEGC = UP * F        # 512 cols per segment
NSEG = COLS // SEGC  # 32 segments per core
KI = 2 * F + 1       # 65: interp-block contraction size
NBIAS = 8 + 4 + 2 + 4 + 8  # packed bias columns
W6SCALE = 32.0       # fp8 pre-scale on w6 / interp coeffs; descaled at PSUM

_CACHE: dict = {}
_SIM_NOCC = False  # sim-only: replace the collective with a local DMA copy
_USE_DR = True     # debug: False = plain fp8 matmuls (K=128) instead of DoubleRow


def _build_program(reps=1, phase="all"):
    """One SPMD program; per-core data differences live in the inputs.

    reps>1 repeats the whole body back-to-back inside one NEFF — used only
    by the timing harness (per-exec time = wall-time delta / extra reps).
    phase: "all" | "enc" (encoder only) | "big" (projection only, dummy d2).
    """
    bf16, f32, fp8 = DT.bfloat16, DT.float32, DT.float8e4
    DR = mybir.MatmulPerfMode.DoubleRow
    nc = bacc.Bacc("TRN2", target_bir_lowering=False, debug=False, num_devices=NCORES)

    KSH = LF // NCORES  # 1024 contraction rows of layer 1 per core
    xTs = nc.dram_tensor("xTs", [KSH, B], bf16, kind="ExternalInput")
    w1s = nc.dram_tensor("w1s", [KSH, 2 * HID], bf16, kind="ExternalInput")
    arin = nc.dram_tensor("arin", [2 * HID, B], bf16)
    arout = nc.dram_tensor("arout", [2 * HID, B], bf16, addr_space="Shared")
    w2 = nc.dram_tensor("w2", [2 * HID, HID], bf16, kind="ExternalInput")
    w3 = nc.dram_tensor("w3", [HID, HID // 2], bf16, kind="ExternalInput")
    w4 = nc.dram_tensor("w4", [HID // 2, HID], bf16, kind="ExternalInput")
    w5 = nc.dram_tensor("w5", [HID, 2 * HID], bf16, kind="ExternalInput")
    bpk = nc.dram_tensor("bpk", [128, NBIAS], f32, kind="ExternalInput")
    w6e = nc.dram_tensor("w6e", [2 * HID, COLS], fp8, kind="ExternalInput")
    ipk = nc.dram_tensor("ipk", [NSEG, KI, 2, SEGC], bf16, kind="ExternalInput")
    out = nc.dram_tensor("out", [B, COLS], bf16, kind="ExternalOutput")

    RELU = mybir.ActivationFunctionType.Relu
    IDENT = mybir.ActivationFunctionType.Identity

    SGRP = 2  # segments per output-staging group
    PSTG = 16  # segments whose interp matmul is pre-staged during the AllReduce

    with tile.TileContext(nc) as tc:

        def _one_rep(rep, ctx):
            psum = ctx.enter_context(
                tc.tile_pool(name=f"psum{rep}", bufs=8, space="PSUM")
            )
            xpool = ctx.enter_context(tc.tile_pool(name=f"xpool{rep}", bufs=4))
            wpool = ctx.enter_context(tc.tile_pool(name=f"wpool{rep}", bufs=5))
            scratch = ctx.enter_context(tc.tile_pool(name=f"scratch{rep}", bufs=2))
            acts = ctx.enter_context(tc.tile_pool(name=f"acts{rep}", bufs=1))
            bpool = ctx.enter_context(tc.tile_pool(name=f"bpool{rep}", bufs=1))
            w6pool = ctx.enter_context(tc.tile_pool(name=f"w6pool{rep}", bufs=3))
            ipool = ctx.enter_context(tc.tile_pool(name=f"ipool{rep}", bufs=2))
            opool = ctx.enter_context(tc.tile_pool(name=f"opool{rep}", bufs=2))
            stpool = ctx.enter_context(tc.tile_pool(name=f"stpool{rep}", bufs=1))

            btile = bpool.tile([128, NBIAS], f32, name="btile")
            nc.scalar.dma_start(btile[:], bpk[:])
            boff = {1: 0, 2: 8, 3: 12, 4: 14, 5: 18}

            def _enc():
                # ---- L1: K-sharded partial matmul + AllReduce over 8 cores ----
                ps1 = [
                    psum.tile([128, B], f32, tag="psum", name=f"ps1_{m}")
                    for m in range(8)
                ]
                for kc in range(4):
                    e1 = nc.sync if kc % 2 == 0 else nc.scalar
                    e2 = nc.scalar if kc % 2 == 0 else nc.sync
                    xt = xpool.tile([128, 2, B], bf16, name=f"xt{kc}", tag="xt")
                    e2.dma_start(
                        xt[:],
                        xTs[ds(kc * 256, 256), :].rearrange("(k p) d -> p k d", p=128),
                    )
                    w1t = wpool.tile([128, 2, 2 * HID], bf16, name=f"w1t{kc}", tag="w")
                    e1.dma_start(
                        w1t[:],
                        w1s[ds(kc * 256, 256), :].rearrange("(k p) d -> p k d", p=128),
                    )
                    for k4 in range(2):
                        for m in range(8):
                            nc.tensor.matmul(
                                ps1[m][:],
                                w1t[:, k4, ts(m, 128)],
                                xt[:, k4, :],
                                start=(kc == 0 and k4 == 0),
                                stop=(kc == 3 and k4 == 1),
                            )
                hp = scratch.tile([128, 8, B], bf16, tag="s", name="hp")
                for m in range(8):
                    nc.vector.tensor_copy(hp[:, m, :], ps1[m][:])
                nc.sync.dma_start(
                    arin[ds(0, 512), :].rearrange("(m p) d -> p m d", p=128),
                    hp[:, 0:4, :],
                )
                nc.scalar.dma_start(
                    arin[ds(512, 512), :].rearrange("(m p) d -> p m d", p=128),
                    hp[:, 4:8, :],
                )
                if _SIM_NOCC:
                    nc.gpsimd.dma_start(arout[:], arin[:])
                else:
                    nc.gpsimd.collective_compute(
                        "AllReduce",
                        mybir.AluOpType.add,
                        replica_groups=[list(range(NCORES))],
                        ins=[arin[:]],
                        outs=[arout[:]],
                    )
                # ---- fill the AllReduce wait (~60us measured end-to-end, PE
                # otherwise idle + HAM-cold) with the first PSTG segments'
                # interp matmuls, staged to SBUF.  Their big-phase chains then
                # start on the first fp8 matmul and re-add ibase at evacuation.
                ibase = []
                for s in range(PSTG):
                    ipx = ipool.tile([KI, 2, SEGC], bf16, name=f"ipx{s}", tag="ip")
                    nc.scalar.dma_start(ipx[:], ipk[s])
                    ib = stpool.tile(
                        [128, 4, SEGC], bf16, name=f"ib{s}", tag=f"ib{s}"
                    )
                    psi = [
                        psum.tile([128, SEGC], f32, tag="psum", name=f"psi_{s}_{b}")
                        for b in range(4)
                    ]
                    for b in range(4):
                        nc.tensor.matmul(
                            psi[b][:],
                            ipx[:, 0, ts(b, 128)],
                            ipx[:, 1, :],
                            start=True,
                            stop=True,
                        )
                    for b in range(4):
                        nc.vector.tensor_scalar_mul(
                            ib[:, b, :], psi[b][:], 1.0 / W6SCALE
                        )
                    ibase.append(ib)

                htmp = scratch.tile([128, 8, B], bf16, tag="s", name="htmp")
                nc.sync.dma_start(
                    htmp[:, 0:4, :],
                    arout[ds(0, 512), :].rearrange("(m p) d -> p m d", p=128),
                )
                nc.scalar.dma_start(
                    htmp[:, 4:8, :],
                    arout[ds(512, 512), :].rearrange("(m p) d -> p m d", p=128),
                )
                h1 = scratch.tile([128, 8, B], bf16, tag="s", name="h1")
                for m in range(8):
                    nc.scalar.activation(
                        h1[:, m, :], htmp[:, m, :], RELU, bias=btile[:, m : m + 1]
                    )

                # ---- L2..L5 (one DMA per layer, weights via shared pool) ----
                def mlp_layer(w_dram, k_tiles, m_tiles, rhs, b_idx, func, name, pool,
                              out_dtype=bf16):
                    o = pool.tile(
                        [128, m_tiles, B], out_dtype,
                        tag="s" if pool is scratch else name, name=name,
                    )
                    ps = [
                        psum.tile([128, B], f32, tag="psum", name=f"ps_{name}_{m}")
                        for m in range(m_tiles)
                    ]
                    for kc in range(0, k_tiles, 2):
                        kw = min(2, k_tiles - kc)
                        wt = wpool.tile(
                            [128, kw, m_tiles * 128], bf16, tag="w",
                            name=f"w_{name}_{kc}",
                        )
                        eng = nc.sync if (kc // 2) % 2 == 0 else nc.scalar
                        eng.dma_start(
                            wt[:],
                            w_dram[ds(kc * 128, kw * 128), :].rearrange(
                                "(k p) d -> p k d", p=128
                            ),
                        )
                        for ki in range(kw):
                            for m in range(m_tiles):
                                nc.tensor.matmul(
                                    ps[m][:],
                                    wt[:, ki, ts(m, 128)],
                                    rhs[:, kc + ki, :],
                                    start=(kc + ki == 0),
                                    stop=(kc + ki == k_tiles - 1),
                                )
                    ob = boff[b_idx]
                    for m in range(m_tiles):
                        nc.scalar.activation(
                            o[:, m, :], ps[m][:], func,
                            bias=btile[:, ob + m : ob + m + 1],
                        )
                    return o

                h2 = mlp_layer(w2, 8, 4, h1, 2, RELU, "h2", scratch)
                ft = mlp_layer(w3, 4, 2, h2, 3, IDENT, "ft", scratch)
                d1 = mlp_layer(w4, 2, 4, ft, 4, RELU, "d1", scratch)
                d2 = mlp_layer(w5, 4, 8, d1, 5, RELU, "d2", acts, out_dtype=fp8)
                return d2, ibase

            def _big(d2, ibase):
                # ---- big projection + folded constraint, 32 segments ----
                # Per 512-col segment and batch tile b: one PSUM bank, two
                # independent 256-col accumulation chains (DoubleRow needs
                # N<=256): bf16 interp matmul (start) + 4 fp8 K=256 matmuls.
                for g in range(NSEG // SGRP):
                    ot = [
                        opool.tile(
                            [128, SGRP, SEGC], bf16, tag=f"ot{b}", name=f"ot{g}_{b}"
                        )
                        for b in range(4)
                    ]
                    for si in range(SGRP):
                        s = g * SGRP + si
                        staged = s < len(ibase)
                        if not staged:
                            ip = ipool.tile(
                                [KI, 2, SEGC], bf16, name=f"ip{s}", tag="ip"
                            )
                            nc.scalar.dma_start(ip[:], ipk[s])
                        w6t = w6pool.tile(
                            [128, 8, SEGC], fp8, name=f"w6t{s}", tag="w6"
                        )
                        nc.sync.dma_start(
                            w6t[:, 0:4, :],
                            w6e[ds(0, 512), ts(s, SEGC)].rearrange(
                                "(k p) d -> p k d", p=128
                            ),
                        )
                        nc.scalar.dma_start(
                            w6t[:, 4:8, :],
                            w6e[ds(512, 512), ts(s, SEGC)].rearrange(
                                "(k p) d -> p k d", p=128
                            ),
                        )
                        pso = [
                            psum.tile([128, SEGC], f32, tag="psum", name=f"pso_{s}_{b}")
                            for b in range(4)
                        ]
                        if staged:
                            # interp already in SBUF: full-width DR chain, the
                            # kp=0 matmul zeroes the bank (full-bank start, so
                            # no per-half clobber); ibase re-added at evac.
                            for kp in range(4):
                                for b in range(4):
                                    nc.tensor.matmul(
                                        pso[b][:],
                                        d2[:, ds(2 * kp, 2), ts(b, 128)],
                                        w6t[:, ds(2 * kp, 2), :],
                                        start=(kp == 0),
                                        stop=(kp == 3),
                                        perf_mode=DR,
                                    )
                            for b in range(4):
                                nc.vector.scalar_tensor_tensor(
                                    ot[b][:, si, :],
                                    pso[b][:],
                                    1.0 / W6SCALE,
                                    ibase[s][:, b, :],
                                    mybir.AluOpType.mult,
                                    mybir.AluOpType.add,
                                )
                            continue
                        for b in range(4):
                            # one full-width start per PSUM bank: HW zeroes at
                            # bank granularity, so per-half starts would clobber
                            nc.tensor.matmul(
                                pso[b][:],
                                ip[:, 0, ts(b, 128)],
                                ip[:, 1, :],
                                start=True,
                                stop=False,
                            )
                        if _USE_DR:
                            for kp in range(4):
                                for b in range(4):
                                    for nh in range(2):
                                        nc.tensor.matmul(
                                            pso[b][:, ds(nh * 256, 256)],
                                            d2[:, ds(2 * kp, 2), ts(b, 128)],
                                            w6t[:, ds(2 * kp, 2), ds(nh * 256, 256)],
                                            start=False,
                                            stop=(kp == 3),
                                            perf_mode=DR,
                                        )
                        else:
                            for ki in range(8):
                                for b in range(4):
                                    nc.tensor.matmul(
                                        pso[b][:],
                                        d2[:, ki, ts(b, 128)],
                                        w6t[:, ki, :],
                                        start=False,
                                        stop=(ki == 7),
                                    )
                        for b in range(4):
                            nc.vector.tensor_scalar_mul(
                                ot[b][:, si, :], pso[b][:], 1.0 / W6SCALE
                            )
                    for b in range(4):
                        eng = nc.sync if b % 2 == 0 else nc.scalar
                        eng.dma_start(
                            out[ts(b, 128), ds(g * SGRP * SEGC, SGRP * SEGC)].rearrange(
                                "p (a x) -> p a x", a=SGRP
                            ),
                            ot[b][:],
                        )

            if phase == "enc":
                d2, _ib = _enc()
                otx = opool.tile([128, B], bf16, name="otx", tag="ot0")
                nc.vector.tensor_copy(otx[:], d2[:, 0, :])
                nc.sync.dma_start(out[ts(0, 128), ts(0, B)], otx[:])
            elif phase == "big":
                d2 = acts.tile([128, 8, B], fp8, tag="d2", name="d2")
                nc.vector.memset(d2[:].bitcast(DT.uint32), 1)
                _big(d2, [])
            else:
                d2, ibase = _enc()
                _big(d2, ibase)

        for _rep in range(reps):
            with ExitStack() as _ctx:
                _one_rep(_rep, _ctx)

    nc.compile()
    return nc


def _host_prep(inputs):
    """Shard + fold. Returns per-core input maps."""
    x = np.ascontiguousarray(inputs["low_res_data"], dtype=np.float32)
    x2d = x.reshape(B, LF)
    xTa = np.ascontiguousarray(x2d.T)
    w6 = np.asarray(inputs["w6"], dtype=np.float32)
    b6 = np.asarray(inputs["b6"], dtype=np.float32)

    # per-output-column scale: 0 on knots, 0.2 in-segment, 1.0 in the tail
    h = np.arange(H)
    colscale = np.where(h % UP == 0, 0.0, np.where(h < (L - 1) * UP, 0.2, 1.0))
    colscale = np.repeat(colscale, F).astype(np.float32)  # (HF,)
    b6_eff = b6 * colscale

    # interp coefficient blocks (shared by all segments except the last);
    # coefficient side carries the W6SCALE fp8 pre-scale.
    fidx = np.arange(F)
    std = np.zeros((KI, SEGC), np.float32)
    last = np.zeros((KI, SEGC), np.float32)
    for h_off in range(UP):
        a = h_off / UP
        cs = 1.0 if h_off == 0 else 0.8 * (1.0 - a)
        ce = 0.0 if h_off == 0 else 0.8 * a
        std[fidx, h_off * F + fidx] = cs * W6SCALE
        last[fidx, h_off * F + fidx] = W6SCALE if h_off == 0 else 0.0
        std[F + fidx, h_off * F + fidx] = ce * W6SCALE

    bpk = np.zeros((128, NBIAS), np.float32)
    off = 0
    for i in (1, 2, 3, 4, 5):
        bv = np.asarray(inputs[f"b{i}"], np.float32)
        m = bv.shape[0] // 128
        bpk[:, off : off + m] = bv.reshape(m, 128).T
        off += m

    w1f = np.asarray(inputs["w1"], np.float32)
    shared = {
        "w2": np.asarray(inputs["w2"], np.float32).astype(bfloat16),
        "w3": np.asarray(inputs["w3"], np.float32).astype(bfloat16),
        "w4": np.asarray(inputs["w4"], np.float32).astype(bfloat16),
        "w5": np.asarray(inputs["w5"], np.float32).astype(bfloat16),
        "bpk": bpk,
    }

    in_maps = []
    for c in range(NCORES):
        j0 = c * COLS
        w6e = np.ascontiguousarray(
            w6[:, j0 : j0 + COLS] * colscale[j0 : j0 + COLS] * W6SCALE
        ).astype(float8_e4m3)
        xTsl = np.ascontiguousarray(
            xTa[c * (LF // NCORES) : (c + 1) * (LF // NCORES)]
        ).astype(bfloat16)
        w1sl = np.ascontiguousarray(
            w1f[c * (LF // NCORES) : (c + 1) * (LF // NCORES)]
        ).astype(bfloat16)
        ipk = np.zeros((NSEG, KI, 2, SEGC), np.float32)
        for sl in range(NSEG):
            s = c * NSEG + sl
            ipk[sl, 0:F, 0] = xTa[s * F : (s + 1) * F]
            if s + 1 < L:
                ipk[sl, F : 2 * F, 0] = xTa[(s + 1) * F : (s + 2) * F]
            ipk[sl, 2 * F, 0] = 1.0
            ipk[sl, :, 1] = std if s < L - 1 else last
            ipk[sl, 2 * F, 1] = b6_eff[s * SEGC : (s + 1) * SEGC] * W6SCALE
        in_maps.append(
            {
                **shared,
                "w6e": w6e,
                "ipk": ipk.astype(bfloat16),
                "xTs": xTsl,
                "w1s": w1sl,
            }
        )
    return in_maps


def kernel(**inputs):
    if "nc" not in _CACHE:
        _CACHE["nc"] = _build_program()
    nc = _CACHE["nc"]
    in_maps = _host_prep(inputs)
    res = run_bass_kernel_spmd(nc, in_maps, list(range(NCORES)))
    out = np.empty((B, H, F), np.float32)
    for c in range(NCORES):
        out[:, c * (H // NCORES) : (c + 1) * (H // NCORES), :] = (
            res.results[c]["out"].astype(np.float32).reshape(B, H // NCORES, F)
        )
    # knot timesteps are exact copies of the low-res input; write them in f32
    out[:, ::UP, :] = np.asarray(inputs["low_res_data"], np.float32)
    return out


# revision 45
# speedup vs baseline: 1.0891x; 1.0891x over previous
"""Trainium2 Bass kernel for nn_ConstrainedEnhancementModel.

Model: x(512,256,32) -> flatten(512,8192) -> MLP encoder/decoder
(8192->1024->512->256->512->1024->131072) -> constraint blend with
linearly-interpolated low-res data.

Key transformation (host-side fold): the post-matmul constraint
   out = where(knot, x_knot, where(in_seg, 0.8*interp + 0.2*decoded, decoded))
is linear in x, so it folds into the final projection:
 - scale w6/b6 columns by 0 (knot), 0.2 (in-segment) or 1.0 (tail)
 - per 512-column segment, append a K=65 matmul block:
   32 rows of x (segment start vals), 32 rows (segment end vals), and a
   ones-row carrying the bias b6_eff.
Device kernel is then a pure matmul pipeline (no gather/select on chip).

Precision plan (rel-err budget 2e-2): the dominant projection runs in
fp8 e4m3 with MatmulPerfMode.DoubleRow (K=256/instr, 2x PE rate).  w6 is
pre-scaled by 32 so its N(0, 1/32) entries use e4m3's normal range; the
interp coefficient block is pre-scaled by 32 in bf16, and the PSUM is
descaled by exactly 1/32 during evacuation.  decoded carries only 0.2
weight in most of the output, so its ~4% fp8 error contributes ~1%.
Encoder runs in bf16 end-to-end (including the AllReduce payload).
Output is written bf16 and upcast on host; knot columns are overwritten
host-side with exact f32 input values.

Sharding over 8 cores: encoder L1 is K-sharded + AllReduce at full
batch in transposed layout [features, batch]; L2-L5 replicated; the
1024x131072 projection is tensor-parallel column-sharded (16384 cols =
512 output timesteps per core); core c writes output rows [c*512,
(c+1)*512) of H.

Perf notes: DMA instruction issue costs ~0.6-1us of sequencer time, so
everything is loaded with few, large multi-dim-AP DMAs split across the
two HWDGE-capable engines (sync, scalar).
"""

from contextlib import ExitStack

import numpy as np
from ml_dtypes import bfloat16, float8_e4m3

import concourse.bacc as bacc
import concourse.mybir as mybir
import concourse.tile as tile
from concourse.bass import ds, ts
from concourse.bass_utils import run_bass_kernel_spmd

DT = mybir.dt

B, L, F, H, HID = 512, 256, 32, 4096, 512
UP = H // L          # 16 timesteps per low-res segment
LF = L * F           # 8192
HF = H * F           # 131072
NCORES = 8
COLS = HF // NCORES  # 16384 output cols per core
SEGC = UP * F        # 512 cols per segment
NSEG = COLS // SEGC  # 32 segments per core
KI = 2 * F + 1       # 65: interp-block contraction size
NBIAS = 8 + 4 + 2 + 4 + 8  # packed bias columns
W6SCALE = 32.0       # fp8 pre-scale on w6 / interp coeffs; descaled at PSUM

_CACHE: dict = {}
_SIM_NOCC = False  # sim-only: replace the collective with a local DMA copy
_USE_DR = True     # debug: False = plain fp8 matmuls (K=128) instead of DoubleRow


def _build_program(reps=1, phase="all"):
    """One SPMD program; per-core data differences live in the inputs.

    reps>1 repeats the whole body back-to-back inside one NEFF — used only
    by the timing harness (per-exec time = wall-time delta / extra reps).
    phase: "all" | "enc" (encoder only) | "big" (projection only, dummy d2).
    """
    bf16, f32, fp8 = DT.bfloat16, DT.float32, DT.float8e4
    DR = mybir.MatmulPerfMode.DoubleRow
    nc = bacc.Bacc("TRN2", target_bir_lowering=False, debug=False, num_devices=NCORES)

    KSH = LF // NCORES  # 1024 contraction rows of layer 1 per core
    xTs = nc.dram_tensor("xTs", [KSH, B], bf16, kind="ExternalInput")
    w1s = nc.dram_tensor("w1s", [KSH, 2 * HID], bf16, kind="ExternalInput")
    arin = nc.dram_tensor("arin", [2 * HID, B], bf16)
    arout = nc.dram_tensor("arout", [2 * HID, B], bf16, addr_space="Shared")
    w2 = nc.dram_tensor("w2", [2 * HID, HID], bf16, kind="ExternalInput")
    w3 = nc.dram_tensor("w3", [HID, HID // 2], bf16, kind="ExternalInput")
    w4 = nc.dram_tensor("w4", [HID // 2, HID], bf16, kind="ExternalInput")
    w5 = nc.dram_tensor("w5", [HID, 2 * HID], bf16, kind="ExternalInput")
    bpk = nc.dram_tensor("bpk", [128, NBIAS], f32, kind="ExternalInput")
    w6e = nc.dram_tensor("w6e", [2 * HID, COLS], fp8, kind="ExternalInput")
    ipk = nc.dram_tensor("ipk", [NSEG, KI, 2, SEGC], bf16, kind="ExternalInput")
    out = nc.dram_tensor("out", [B, COLS], bf16, kind="ExternalOutput")

    RELU = mybir.ActivationFunctionType.Relu
    IDENT = mybir.ActivationFunctionType.Identity

    SGRP = 2  # segments per output-staging group
    PSTG = 10  # segments whose interp matmul is pre-staged during the AllReduce

    with tile.TileContext(nc) as tc:

        def _one_rep(rep, ctx):
            psum = ctx.enter_context(
                tc.tile_pool(name=f"psum{rep}", bufs=8, space="PSUM")
            )
            xpool = ctx.enter_context(tc.tile_pool(name=f"xpool{rep}", bufs=4))
            wpool = ctx.enter_context(tc.tile_pool(name=f"wpool{rep}", bufs=5))
            scratch = ctx.enter_context(tc.tile_pool(name=f"scratch{rep}", bufs=2))
            acts = ctx.enter_context(tc.tile_pool(name=f"acts{rep}", bufs=1))
            bpool = ctx.enter_context(tc.tile_pool(name=f"bpool{rep}", bufs=1))
            w6pool = ctx.enter_context(tc.tile_pool(name=f"w6pool{rep}", bufs=3))
            ipool = ctx.enter_context(tc.tile_pool(name=f"ipool{rep}", bufs=2))
            opool = ctx.enter_context(tc.tile_pool(name=f"opool{rep}", bufs=2))
            stpool = ctx.enter_context(tc.tile_pool(name=f"stpool{rep}", bufs=1))

            btile = bpool.tile([128, NBIAS], f32, name="btile")
            nc.scalar.dma_start(btile[:], bpk[:])
            boff = {1: 0, 2: 8, 3: 12, 4: 14, 5: 18}

            def _enc():
                # ---- L1: K-sharded partial matmul + AllReduce over 8 cores ----
                ps1 = [
                    psum.tile([128, B], f32, tag="psum", name=f"ps1_{m}")
                    for m in range(8)
                ]
                for kc in range(4):
                    e1 = nc.sync if kc % 2 == 0 else nc.scalar
                    e2 = nc.scalar if kc % 2 == 0 else nc.sync
                    xt = xpool.tile([128, 2, B], bf16, name=f"xt{kc}", tag="xt")
                    e2.dma_start(
                        xt[:],
                        xTs[ds(kc * 256, 256), :].rearrange("(k p) d -> p k d", p=128),
                    )
                    w1t = wpool.tile([128, 2, 2 * HID], bf16, name=f"w1t{kc}", tag="w")
                    e1.dma_start(
                        w1t[:],
                        w1s[ds(kc * 256, 256), :].rearrange("(k p) d -> p k d", p=128),
                    )
                    for k4 in range(2):
                        for m in range(8):
                            nc.tensor.matmul(
                                ps1[m][:],
                                w1t[:, k4, ts(m, 128)],
                                xt[:, k4, :],
                                start=(kc == 0 and k4 == 0),
                                stop=(kc == 3 and k4 == 1),
                            )
                hp = scratch.tile([128, 8, B], bf16, tag="s", name="hp")
                for m in range(8):
                    nc.vector.tensor_copy(hp[:, m, :], ps1[m][:])
                nc.sync.dma_start(
                    arin[ds(0, 512), :].rearrange("(m p) d -> p m d", p=128),
                    hp[:, 0:4, :],
                )
                nc.scalar.dma_start(
                    arin[ds(512, 512), :].rearrange("(m p) d -> p m d", p=128),
                    hp[:, 4:8, :],
                )
                if _SIM_NOCC:
                    nc.gpsimd.dma_start(arout[:], arin[:])
                else:
                    nc.gpsimd.collective_compute(
                        "AllReduce",
                        mybir.AluOpType.add,
                        replica_groups=[list(range(NCORES))],
                        ins=[arin[:]],
                        outs=[arout[:]],
                    )
                # ---- fill the AllReduce wait (~60us measured end-to-end, PE
                # otherwise idle + HAM-cold) with the first PSTG segments'
                # interp matmuls, staged to SBUF.  Their big-phase chains then
                # start on the first fp8 matmul and re-add ibase at evacuation.
                ibase = []
                for s in range(PSTG):
                    ipx = ipool.tile([KI, 2, SEGC], bf16, name=f"ipx{s}", tag="ip")
                    nc.scalar.dma_start(ipx[:], ipk[s])
                    ib = stpool.tile(
                        [128, 4, SEGC], bf16, name=f"ib{s}", tag=f"ib{s}"
                    )
                    psi = [
                        psum.tile([128, SEGC], f32, tag="psum", name=f"psi_{s}_{b}")
                        for b in range(4)
                    ]
                    for b in range(4):
                        nc.tensor.matmul(
                            psi[b][:],
                            ipx[:, 0, ts(b, 128)],
                            ipx[:, 1, :],
                            start=True,
                            stop=True,
                        )
                    for b in range(4):
                        nc.vector.tensor_scalar_mul(
                            ib[:, b, :], psi[b][:], 1.0 / W6SCALE
                        )
                    ibase.append(ib)

                htmp = scratch.tile([128, 8, B], bf16, tag="s", name="htmp")
                nc.sync.dma_start(
                    htmp[:, 0:4, :],
                    arout[ds(0, 512), :].rearrange("(m p) d -> p m d", p=128),
                )
                nc.scalar.dma_start(
                    htmp[:, 4:8, :],
                    arout[ds(512, 512), :].rearrange("(m p) d -> p m d", p=128),
                )
                h1 = scratch.tile([128, 8, B], bf16, tag="s", name="h1")
                for m in range(8):
                    nc.scalar.activation(
                        h1[:, m, :], htmp[:, m, :], RELU, bias=btile[:, m : m + 1]
                    )

                # ---- L2..L5 (one DMA per layer, weights via shared pool) ----
                def mlp_layer(w_dram, k_tiles, m_tiles, rhs, b_idx, func, name, pool,
                              out_dtype=bf16):
                    o = pool.tile(
                        [128, m_tiles, B], out_dtype,
                        tag="s" if pool is scratch else name, name=name,
                    )
                    ps = [
                        psum.tile([128, B], f32, tag="psum", name=f"ps_{name}_{m}")
                        for m in range(m_tiles)
                    ]
                    for kc in range(0, k_tiles, 2):
                        kw = min(2, k_tiles - kc)
                        wt = wpool.tile(
                            [128, kw, m_tiles * 128], bf16, tag="w",
                            name=f"w_{name}_{kc}",
                        )
                        eng = nc.sync if (kc // 2) % 2 == 0 else nc.scalar
                        eng.dma_start(
                            wt[:],
                            w_dram[ds(kc * 128, kw * 128), :].rearrange(
                                "(k p) d -> p k d", p=128
                            ),
                        )
                        for ki in range(kw):
                            for m in range(m_tiles):
                                nc.tensor.matmul(
                                    ps[m][:],
                                    wt[:, ki, ts(m, 128)],
                                    rhs[:, kc + ki, :],
                                    start=(kc + ki == 0),
                                    stop=(kc + ki == k_tiles - 1),
                                )
                    ob = boff[b_idx]
                    for m in range(m_tiles):
                        nc.scalar.activation(
                            o[:, m, :], ps[m][:], func,
                            bias=btile[:, ob + m : ob + m + 1],
                        )
                    return o

                h2 = mlp_layer(w2, 8, 4, h1, 2, RELU, "h2", scratch)
                ft = mlp_layer(w3, 4, 2, h2, 3, IDENT, "ft", scratch)
                d1 = mlp_layer(w4, 2, 4, ft, 4, RELU, "d1", scratch)
                d2 = mlp_layer(w5, 4, 8, d1, 5, RELU, "d2", acts, out_dtype=fp8)
                return d2, ibase

            def _big(d2, ibase):
                # ---- big projection + folded constraint, 32 segments ----
                # Per 512-col segment and batch tile b: one PSUM bank, two
                # independent 256-col accumulation chains (DoubleRow needs
                # N<=256): bf16 interp matmul (start) + 4 fp8 K=256 matmuls.
                for g in range(NSEG // SGRP):
                    ot = [
                        opool.tile(
                            [128, SGRP, SEGC], bf16, tag=f"ot{b}", name=f"ot{g}_{b}"
                        )
                        for b in range(4)
                    ]
                    for si in range(SGRP):
                        s = g * SGRP + si
                        staged = s < len(ibase)
                        if not staged:
                            ip = ipool.tile(
                                [KI, 2, SEGC], bf16, name=f"ip{s}", tag="ip"
                            )
                            nc.scalar.dma_start(ip[:], ipk[s])
                        w6t = w6pool.tile(
                            [128, 8, SEGC], fp8, name=f"w6t{s}", tag="w6"
                        )
                        nc.sync.dma_start(
                            w6t[:, 0:4, :],
                            w6e[ds(0, 512), ts(s, SEGC)].rearrange(
                                "(k p) d -> p k d", p=128
                            ),
                        )
                        nc.scalar.dma_start(
                            w6t[:, 4:8, :],
                            w6e[ds(512, 512), ts(s, SEGC)].rearrange(
                                "(k p) d -> p k d", p=128
                            ),
                        )
                        pso = [
                            psum.tile([128, SEGC], f32, tag="psum", name=f"pso_{s}_{b}")
                            for b in range(4)
                        ]
                        if staged:
                            # interp already in SBUF: full-width DR chain, the
                            # kp=0 matmul zeroes the bank (full-bank start, so
                            # no per-half clobber); ibase re-added at evac.
                            for kp in range(4):
                                for b in range(4):
                                    nc.tensor.matmul(
                                        pso[b][:],
                                        d2[:, ds(2 * kp, 2), ts(b, 128)],
                                        w6t[:, ds(2 * kp, 2), :],
                                        start=(kp == 0),
                                        stop=(kp == 3),
                                        perf_mode=DR,
                                    )
                            for b in range(4):
                                nc.vector.scalar_tensor_tensor(
                                    ot[b][:, si, :],
                                    pso[b][:],
                                    1.0 / W6SCALE,
                                    ibase[s][:, b, :],
                                    mybir.AluOpType.mult,
                                    mybir.AluOpType.add,
                                )
                            continue
                        for b in range(4):
                            # one full-width start per PSUM bank: HW zeroes at
                            # bank granularity, so per-half starts would clobber
                            nc.tensor.matmul(
                                pso[b][:],
                                ip[:, 0, ts(b, 128)],
                                ip[:, 1, :],
                                start=True,
                                stop=False,
                            )
                        if _USE_DR:
                            for kp in range(4):
                                for b in range(4):
                                    for nh in range(2):
                                        nc.tensor.matmul(
                                            pso[b][:, ds(nh * 256, 256)],
                                            d2[:, ds(2 * kp, 2), ts(b, 128)],
                                            w6t[:, ds(2 * kp, 2), ds(nh * 256, 256)],
                                            start=False,
                                            stop=(kp == 3),
                                            perf_mode=DR,
                                        )
                        else:
                            for ki in range(8):
                                for b in range(4):
                                    nc.tensor.matmul(
                                        pso[b][:],
                                        d2[:, ki, ts(b, 128)],
                                        w6t[:, ki, :],
                                        start=False,
                                        stop=(ki == 7),
                                    )
                        for b in range(4):
                            nc.vector.tensor_scalar_mul(
                                ot[b][:, si, :], pso[b][:], 1.0 / W6SCALE
                            )
                    for b in range(4):
                        eng = nc.sync if b % 2 == 0 else nc.scalar
                        eng.dma_start(
                            out[ts(b, 128), ds(g * SGRP * SEGC, SGRP * SEGC)].rearrange(
                                "p (a x) -> p a x", a=SGRP
                            ),
                            ot[b][:],
                        )

            if phase == "enc":
                d2, _ib = _enc()
                otx = opool.tile([128, B], bf16, name="otx", tag="ot0")
                nc.vector.tensor_copy(otx[:], d2[:, 0, :])
                nc.sync.dma_start(out[ts(0, 128), ts(0, B)], otx[:])
            elif phase == "big":
                d2 = acts.tile([128, 8, B], fp8, tag="d2", name="d2")
                nc.vector.memset(d2[:].bitcast(DT.uint32), 1)
                _big(d2, [])
            else:
                d2, ibase = _enc()
                _big(d2, ibase)

        for _rep in range(reps):
            with ExitStack() as _ctx:
                _one_rep(_rep, _ctx)

    nc.compile()
    return nc


def _host_prep(inputs):
    """Shard + fold. Returns per-core input maps."""
    x = np.ascontiguousarray(inputs["low_res_data"], dtype=np.float32)
    x2d = x.reshape(B, LF)
    xTa = np.ascontiguousarray(x2d.T)
    w6 = np.asarray(inputs["w6"], dtype=np.float32)
    b6 = np.asarray(inputs["b6"], dtype=np.float32)

    # per-output-column scale: 0 on knots, 0.2 in-segment, 1.0 in the tail
    h = np.arange(H)
    colscale = np.where(h % UP == 0, 0.0, np.where(h < (L - 1) * UP, 0.2, 1.0))
    colscale = np.repeat(colscale, F).astype(np.float32)  # (HF,)
    b6_eff = b6 * colscale

    # interp coefficient blocks (shared by all segments except the last);
    # coefficient side carries the W6SCALE fp8 pre-scale.
    fidx = np.arange(F)
    std = np.zeros((KI, SEGC), np.float32)
    last = np.zeros((KI, SEGC), np.float32)
    for h_off in range(UP):
        a = h_off / UP
        cs = 1.0 if h_off == 0 else 0.8 * (1.0 - a)
        ce = 0.0 if h_off == 0 else 0.8 * a
        std[fidx, h_off * F + fidx] = cs * W6SCALE
        last[fidx, h_off * F + fidx] = W6SCALE if h_off == 0 else 0.0
        std[F + fidx, h_off * F + fidx] = ce * W6SCALE

    bpk = np.zeros((128, NBIAS), np.float32)
    off = 0
    for i in (1, 2, 3, 4, 5):
        bv = np.asarray(inputs[f"b{i}"], np.float32)
        m = bv.shape[0] // 128
        bpk[:, off : off + m] = bv.reshape(m, 128).T
        off += m

    w1f = np.asarray(inputs["w1"], np.float32)
    shared = {
        "w2": np.asarray(inputs["w2"], np.float32).astype(bfloat16),
        "w3": np.asarray(inputs["w3"], np.float32).astype(bfloat16),
        "w4": np.asarray(inputs["w4"], np.float32).astype(bfloat16),
        "w5": np.asarray(inputs["w5"], np.float32).astype(bfloat16),
        "bpk": bpk,
    }

    in_maps = []
    for c in range(NCORES):
        j0 = c * COLS
        w6e = np.ascontiguousarray(
            w6[:, j0 : j0 + COLS] * colscale[j0 : j0 + COLS] * W6SCALE
        ).astype(float8_e4m3)
        xTsl = np.ascontiguousarray(
            xTa[c * (LF // NCORES) : (c + 1) * (LF // NCORES)]
        ).astype(bfloat16)
        w1sl = np.ascontiguousarray(
            w1f[c * (LF // NCORES) : (c + 1) * (LF // NCORES)]
        ).astype(bfloat16)
        ipk = np.zeros((NSEG, KI, 2, SEGC), np.float32)
        for sl in range(NSEG):
            s = c * NSEG + sl
            ipk[sl, 0:F, 0] = xTa[s * F : (s + 1) * F]
            if s + 1 < L:
                ipk[sl, F : 2 * F, 0] = xTa[(s + 1) * F : (s + 2) * F]
            ipk[sl, 2 * F, 0] = 1.0
            ipk[sl, :, 1] = std if s < L - 1 else last
            ipk[sl, 2 * F, 1] = b6_eff[s * SEGC : (s + 1) * SEGC] * W6SCALE
        in_maps.append(
            {
                **shared,
                "w6e": w6e,
                "ipk": ipk.astype(bfloat16),
                "xTs": xTsl,
                "w1s": w1sl,
            }
        )
    return in_maps


def kernel(**inputs):
    if "nc" not in _CACHE:
        _CACHE["nc"] = _build_program()
    nc = _CACHE["nc"]
    in_maps = _host_prep(inputs)
    res = run_bass_kernel_spmd(nc, in_maps, list(range(NCORES)))
    out = np.empty((B, H, F), np.float32)
    for c in range(NCORES):
        out[:, c * (H // NCORES) : (c + 1) * (H // NCORES), :] = (
            res.results[c]["out"].astype(np.float32).reshape(B, H // NCORES, F)
        )
    # knot timesteps are exact copies of the low-res input; write them in f32
    out[:, ::UP, :] = np.asarray(inputs["low_res_data"], np.float32)
    return out
